# revision 10
# baseline (speedup 1.0000x reference)
"""Trainium2 Bass kernel for nn_Block_1726576855578 (dense_mlp).

Sharding: 8-way data parallel over batch B=4096 (512 rows/core), all
weights replicated. Per-core pipeline (all layouts [feature-partition,
batch-free] so chunk ops stay partition-aligned):

  stage1: hT[mm, b] = W.T-tiled matmuls vs xT, bias via K=1 rank-update,
          evicted per-chunk into [81, 512] tiles with a ones row (row 80)
          so the chunk-linear bias rides in the K=81 matmul.
  stage2: per chunk c, t-tile j: y0/y1 = [120(t), 512(b)] psum matmuls,
          evicted to bf16, w = y0*y1 on DVE, rank-sum via 0/1 selector
          matmul accumulating z[80(s), 512(b)] in psum.
  merge:  signed-sqrt via ACT Abs/Sign/Sqrt + DVE mul; chunk L2 norm via
          ones-matmul on |z| (sum z_signed^2 == sum |z| exactly),
          1/max(sqrt, eps) on a [1,512] row, broadcast back with a K=1
          matmul, applied on DVE.
  stage3: out[b, o] psum accumulation over 20 chunk K-tiles (K=80) plus
          K=1 bias rank-update; evict + DMA.

Matmuls run as float32r (full PE rate at N>=256). The only bf16 in the
pipeline is the y0*y1 elementwise product path.
"""

import numpy as np

import concourse.bacc as bacc
import concourse.mybir as mybir
import concourse.tile as tile
from concourse import bass_utils
from concourse.bass import ts

F32 = mybir.dt.float32
F32R = mybir.dt.float32r
BF16 = mybir.dt.bfloat16
AF = mybir.ActivationFunctionType

NCORES = 8
B = 4096
BC = B // NCORES          # 512 rows per core
D = 2048                  # D0 == D1
MM = 1600
CHUNKS = 20
SIZE = 80                 # mm chunk width
RANK = 15
TDIM = SIZE * RANK        # 1200
TT = 120                  # t-tile width (10 tiles per chunk)
NTT = TDIM // TT          # 10
OUT = 3000
NO = 500                  # out free tile
NNT = OUT // NO           # 6
NBT = BC // 128           # 4 b-tiles
KD = D // 128             # 16 K-tiles over D
EPS = 1e-12

_NC = None


def _build_nc():
    nc = bacc.Bacc("TRN2", target_bir_lowering=False, debug=False,
                   num_devices=NCORES)

    x0t = nc.dram_tensor("x0t", [D, BC], F32R, kind="ExternalInput")
    x1t = nc.dram_tensor("x1t", [D, BC], F32R, kind="ExternalInput")
    w0t = nc.dram_tensor("w0t", [D, MM], F32R, kind="ExternalInput")
    w1t = nc.dram_tensor("w1t", [D, MM], F32R, kind="ExternalInput")
    mw0p = nc.dram_tensor("mw0p", [CHUNKS, SIZE + 1, TDIM], F32R,
                          kind="ExternalInput")
    mw1p = nc.dram_tensor("mw1p", [CHUNKS, SIZE + 1, TDIM], F32R,
                          kind="ExternalInput")
    wot = nc.dram_tensor("wot", [MM, OUT], F32R, kind="ExternalInput")
    b0r = nc.dram_tensor("b0r", [1, MM], F32R, kind="ExternalInput")
    b1r = nc.dram_tensor("b1r", [1, MM], F32R, kind="ExternalInput")
    boutr = nc.dram_tensor("boutr", [1, OUT], F32R, kind="ExternalInput")
    seld = nc.dram_tensor("seld", [2, TT, SIZE], BF16, kind="ExternalInput")
    onesd = nc.dram_tensor("onesd", [1, BC], F32R, kind="ExternalInput")
    onescol = nc.dram_tensor("onescol", [128, 1], F32R, kind="ExternalInput")
    outd = nc.dram_tensor("out", [BC, OUT], F32, kind="ExternalOutput")

    xdr = [x0t, x1t]
    wdr = [w0t, w1t]
    bdr = [b0r, b1r]
    mwdr = [mw0p, mw1p]

    with tile.TileContext(nc) as tc:
        with (
            tc.tile_pool(name="const", bufs=1) as cpool,
            tc.tile_pool(name="hpool", bufs=1) as hpool,
        ):
            ones512 = cpool.tile([1, BC], F32R, tag="ones512")
            nc.sync.dma_start(ones512[:], onesd[:])
            ones80 = cpool.tile([SIZE, 1], F32R, tag="ones80")
            nc.sync.dma_start(ones80[:], onescol[0:SIZE, :])
            ones1_80 = cpool.tile([1, SIZE], F32R, tag="ones1_80")
            nc.sync.dma_start(ones1_80[:], onesd[:, 0:SIZE])
            ones1_128 = cpool.tile([1, 128], F32R, tag="ones1_128")
            nc.sync.dma_start(ones1_128[:], onesd[:, 0:128])
            sel = [cpool.tile([TT, SIZE], BF16, tag=f"sel{p}", name=f"sel{p}")
                   for p in (0, 1)]
            nc.sync.dma_start(sel[0][:], seld[0])
            nc.sync.dma_start(sel[1][:], seld[1])
            bsb = [cpool.tile([1, MM], F32R, tag=f"b{s}", name=f"b{s}")
                   for s in (0, 1)]
            nc.sync.dma_start(bsb[0][:], b0r[:])
            nc.sync.dma_start(bsb[1][:], b1r[:])
            bosb = cpool.tile([1, OUT], F32R, tag="bo")
            nc.sync.dma_start(bosb[:], boutr[:])

            # ---------------- stage 1: h = x @ W.T + b, as hT chunks ------
            h_tiles = [[None] * CHUNKS, [None] * CHUNKS]
            with (
                tc.tile_pool(name="xpool", bufs=1) as xpool,
                tc.tile_pool(name="wpool", bufs=3) as wpool,
                tc.tile_pool(name="ps1", bufs=1, space="PSUM") as ps1,
            ):
                for side in (0, 1):
                    xsb = xpool.tile([128, KD * BC], F32R, tag="x")
                    for k in range(KD):
                        nc.sync.dma_start(xsb[:, ts(k, BC)],
                                          xdr[side][ts(k, 128), :])
                    for cg in range(4):  # chunk groups of 5
                        pss = [ps1.tile([SIZE, BC], F32, tag=f"s1_{g}", name=f"s1_{g}")
                               for g in range(5)]
                        for k in range(KD):
                            wk = wpool.tile([128, 5 * SIZE], F32R, tag="wk")
                            nc.sync.dma_start(
                                wk[:], wdr[side][ts(k, 128), ts(cg, 5 * SIZE)])
                            for g in range(5):
                                nc.tensor.matmul(
                                    pss[g][:], wk[:, ts(g, SIZE)],
                                    xsb[:, ts(k, BC)],
                                    start=(k == 0), stop=False)
                        for g in range(5):
                            c = cg * 5 + g
                            nc.tensor.matmul(
                                pss[g][:], bsb[side][0:1, ts(c, SIZE)],
                                ones512[:], start=False, stop=True)
                            ht = hpool.tile([SIZE + 1, BC], F32R,
                                            tag=f"h{side}_{c}")
                            nc.any.tensor_copy(ht[0:SIZE, :], pss[g][:])
                            nc.sync.dma_start(ht[SIZE:SIZE + 1, :], onesd[:])
                            h_tiles[side][c] = ht

            # ---------------- stage 2: chunk linears + bilinear merge -----
            zn_tiles = [None] * CHUNKS
            with (
                tc.tile_pool(name="mwpool", bufs=2) as mwpool,
                tc.tile_pool(name="wbpool", bufs=2) as wbpool,
                tc.tile_pool(name="spool", bufs=2) as spool,
                tc.tile_pool(name="rowpool", bufs=2) as rowpool,
                tc.tile_pool(name="ypsum", bufs=2, space="PSUM") as ypsum,
                tc.tile_pool(name="zpsum", bufs=2, space="PSUM") as zpsum,
                tc.tile_pool(name="npsum", bufs=1, space="PSUM") as npsum,
                tc.tile_pool(name="rbpsum", bufs=1, space="PSUM") as rbpsum,
            ):
                for c in range(CHUNKS):
                    mwsb = []
                    for side in (0, 1):
                        m = mwpool.tile([SIZE + 1, TDIM], F32R, tag=f"mw{side}")
                        nc.sync.dma_start(m[:], mwdr[side][c])
                        mwsb.append(m)
                    z_ps = zpsum.tile([SIZE, BC], F32, tag="z")
                    for j in range(NTT):
                        yb = []
                        for side in (0, 1):
                            y = ypsum.tile([TT, BC], F32, tag=f"y{side}")
                            nc.tensor.matmul(
                                y[:], mwsb[side][:, ts(j, TT)],
                                h_tiles[side][c][:],
                                start=True, stop=True)
                            b = wbpool.tile([TT, BC], BF16, tag=f"y{side}b")
                            nc.any.tensor_copy(b[:], y[:])
                            yb.append(b)
                        wb = wbpool.tile([TT, BC], BF16, tag="wb")
                        nc.vector.tensor_mul(wb[:], yb[0][:], yb[1][:])
                        nc.tensor.matmul(z_ps[:], sel[j % 2][:], wb[:],
                                         start=(j == 0), stop=(j == NTT - 1))
                    # signed sqrt + chunk L2 norm
                    a = spool.tile([SIZE, BC], F32R, tag="a")
                    nc.scalar.activation(a[:], z_ps[:], AF.Abs)
                    sg = spool.tile([SIZE, BC], F32, tag="sg")
                    nc.scalar.activation(sg[:], z_ps[:], AF.Sign)
                    sq = spool.tile([SIZE, BC], F32, tag="sq")
                    nc.scalar.activation(sq[:], a[:], AF.Sqrt)
                    zs = spool.tile([SIZE, BC], F32, tag="zs")
                    nc.vector.tensor_mul(zs[:], sg[:], sq[:])
                    n2 = npsum.tile([1, BC], F32, tag="n2")
                    nc.tensor.matmul(n2[:], ones80[:], a[:],
                                     start=True, stop=True)
                    sn = rowpool.tile([1, BC], F32, tag="sn")
                    nc.scalar.activation(sn[:], n2[:], AF.Sqrt)
                    nc.vector.tensor_scalar_max(sn[:], sn[:], EPS)
                    rn = rowpool.tile([1, BC], F32, tag="rn")
                    nc.vector.reciprocal_approx_fast(rn[:], sn[:])
                    rnr = rowpool.tile([1, BC], F32R, tag="rnr")
                    nc.vector.tensor_copy(rnr[:], rn[:])
                    rnb = rbpsum.tile([SIZE, BC], F32, tag="rnb")
                    nc.tensor.matmul(rnb[:], ones1_80[:], rnr[:],
                                     start=True, stop=True)
                    zn = hpool.tile([SIZE, BC], F32R, tag=f"h0_{c}")
                    nc.vector.tensor_mul(zn[:], zs[:], rnb[:])
                    zn_tiles[c] = zn

            # ---------------- stage 3: out = zn @ Wout.T + bout ----------
            with (
                tc.tile_pool(name="wopool", bufs=1) as wopool,
                tc.tile_pool(name="opool", bufs=2) as opool,
                tc.tile_pool(name="ps3", bufs=2, space="PSUM") as ps3,
            ):
                for nt in range(NNT):
                    wo = []
                    for c in range(CHUNKS):
                        w = wopool.tile([SIZE, NO], F32R, tag=f"wo_{c}")
                        nc.sync.dma_start(
                            w[:], wot[ts(c, SIZE), ts(nt, NO)])
                        wo.append(w)
                    for bt in range(NBT):
                        ops = ps3.tile([128, NO], F32, tag="o")
                        for c in range(CHUNKS):
                            nc.tensor.matmul(
                                ops[:], zn_tiles[c][:, ts(bt, 128)],
                                wo[c][:], start=(c == 0), stop=False)
                        nc.tensor.matmul(
                            ops[:], ones1_128[:],
                            bosb[0:1, ts(nt, NO)], start=False, stop=True)
                        osb = opool.tile([128, NO], F32, tag="ob")
                        nc.any.tensor_copy(osb[:], ops[:])
                        nc.sync.dma_start(outd[ts(bt, 128), ts(nt, NO)],
                                          osb[:])

    nc.compile()
    return nc


def _get_nc():
    global _NC
    if _NC is None:
        _NC = _build_nc()
    return _NC


def _prep_inputs(x0, x1, W0, b0, W1, b1, mW0, mb0, mW1, mb1, Wout, bout):
    import ml_dtypes
    f = np.float32
    shared = {
        "w0t": np.ascontiguousarray(W0.T, dtype=f),
        "w1t": np.ascontiguousarray(W1.T, dtype=f),
        "mw0p": np.ascontiguousarray(
            np.concatenate([mW0.transpose(0, 2, 1), mb0[:, None, :]], axis=1),
            dtype=f),
        "mw1p": np.ascontiguousarray(
            np.concatenate([mW1.transpose(0, 2, 1), mb1[:, None, :]], axis=1),
            dtype=f),
        "wot": np.ascontiguousarray(Wout.T, dtype=f),
        "b0r": np.ascontiguousarray(b0.reshape(1, MM), dtype=f),
        "b1r": np.ascontiguousarray(b1.reshape(1, MM), dtype=f),
        "boutr": np.ascontiguousarray(bout.reshape(1, OUT), dtype=f),
    }
    k = np.arange(TT)
    s = np.arange(SIZE)
    selm = np.stack([
        ((40 * p + k[:, None]) % SIZE == s[None, :]) for p in (0, 1)
    ]).astype(ml_dtypes.bfloat16)
    shared["seld"] = selm
    shared["onesd"] = np.ones((1, BC), dtype=f)
    shared["onescol"] = np.ones((128, 1), dtype=f)
    x0t = np.ascontiguousarray(x0.T, dtype=f)
    x1t = np.ascontiguousarray(x1.T, dtype=f)
    in_maps = []
    for c in range(NCORES):
        m = dict(shared)
        m["x0t"] = np.ascontiguousarray(x0t[:, c * BC:(c + 1) * BC])
        m["x1t"] = np.ascontiguousarray(x1t[:, c * BC:(c + 1) * BC])
        in_maps.append(m)
    return in_maps


def kernel(**inputs):
    inputs = {k: np.asarray(v, dtype=np.float32) for k, v in inputs.items()}
    in_maps = _prep_inputs(**inputs)
    nc = _get_nc()
    res = bass_utils.run_bass_kernel_spmd(
        nc, in_maps, core_ids=list(range(NCORES)), trace=False)
    return np.concatenate([res.results[c]["out"] for c in range(NCORES)],
                          axis=0)


# revision 11
# speedup vs baseline: 114.7371x; 114.7371x over previous
"""Trainium2 Bass kernel for nn_Block_1726576855578 (dense_mlp).

Sharding: 8-way data parallel over batch B=4096 (512 rows/core), all
weights replicated. Per-core pipeline (all layouts [feature-partition,
batch-free] so chunk ops stay partition-aligned):

  stage1: hT[mm, b] = W.T-tiled matmuls vs xT, bias via K=1 rank-update,
          evicted per-chunk into [81, 512] tiles with a ones row (row 80)
          so the chunk-linear bias rides in the K=81 matmul.
  stage2: per chunk c, t-tile j: y0/y1 = [120(t), 512(b)] psum matmuls,
          evicted to bf16, w = y0*y1 on DVE, rank-sum via 0/1 selector
          matmul accumulating z[80(s), 512(b)] in psum.
  merge:  signed-sqrt via ACT Abs/Sign/Sqrt + DVE mul; chunk L2 norm via
          ones-matmul on |z| (sum z_signed^2 == sum |z| exactly),
          1/max(sqrt, eps) on a [1,512] row, broadcast back with a K=1
          matmul, applied on DVE.
  stage3: out[b, o] psum accumulation over 20 chunk K-tiles (K=80) plus
          K=1 bias rank-update; evict + DMA.

Matmuls run as float32r (full PE rate at N>=256). The only bf16 in the
pipeline is the y0*y1 elementwise product path.
"""

import numpy as np

import concourse.bacc as bacc
import concourse.mybir as mybir
import concourse.tile as tile
from concourse import bass_utils
from concourse.bass import ts

F32 = mybir.dt.float32
F32R = mybir.dt.float32r
BF16 = mybir.dt.bfloat16
AF = mybir.ActivationFunctionType

NCORES = 8
B = 4096
BC = B // NCORES          # 512 rows per core
D = 2048                  # D0 == D1
MM = 1600
CHUNKS = 20
SIZE = 80                 # mm chunk width
RANK = 15
TDIM = SIZE * RANK        # 1200
TT = 120                  # t-tile width (10 tiles per chunk)
NTT = TDIM // TT          # 10
OUT = 3000
NO = 500                  # out free tile
NNT = OUT // NO           # 6
NBT = BC // 128           # 4 b-tiles
KD = D // 128             # 16 K-tiles over D
EPS = 1e-12

_NC = None


def _build_nc():
    nc = bacc.Bacc("TRN2", target_bir_lowering=False, debug=False,
                   num_devices=NCORES)

    x0t = nc.dram_tensor("x0t", [D, BC], F32R, kind="ExternalInput")
    x1t = nc.dram_tensor("x1t", [D, BC], F32R, kind="ExternalInput")
    w0t = nc.dram_tensor("w0t", [D, MM], F32R, kind="ExternalInput")
    w1t = nc.dram_tensor("w1t", [D, MM], F32R, kind="ExternalInput")
    mw0p = nc.dram_tensor("mw0p", [CHUNKS, SIZE + 1, TDIM], F32R,
                          kind="ExternalInput")
    mw1p = nc.dram_tensor("mw1p", [CHUNKS, SIZE + 1, TDIM], F32R,
                          kind="ExternalInput")
    wot = nc.dram_tensor("wot", [MM, OUT], F32R, kind="ExternalInput")
    b0r = nc.dram_tensor("b0r", [1, MM], F32R, kind="ExternalInput")
    b1r = nc.dram_tensor("b1r", [1, MM], F32R, kind="ExternalInput")
    boutr = nc.dram_tensor("boutr", [1, OUT], F32R, kind="ExternalInput")
    seld = nc.dram_tensor("seld", [2, TT, SIZE], BF16, kind="ExternalInput")
    onesd = nc.dram_tensor("onesd", [1, BC], F32R, kind="ExternalInput")
    onescol = nc.dram_tensor("onescol", [128, 1], F32R, kind="ExternalInput")
    outd = nc.dram_tensor("out", [BC, OUT], F32, kind="ExternalOutput")

    xdr = [x0t, x1t]
    wdr = [w0t, w1t]
    bdr = [b0r, b1r]
    mwdr = [mw0p, mw1p]

    with tile.TileContext(nc) as tc:
        with (
            tc.tile_pool(name="const", bufs=1) as cpool,
            tc.tile_pool(name="hpool", bufs=1) as hpool,
        ):
            ones512 = cpool.tile([1, BC], F32R, tag="ones512")
            nc.sync.dma_start(ones512[:], onesd[:])
            ones80 = cpool.tile([SIZE, 1], F32R, tag="ones80")
            nc.sync.dma_start(ones80[:], onescol[0:SIZE, :])
            ones1_80 = cpool.tile([1, SIZE], F32R, tag="ones1_80")
            nc.sync.dma_start(ones1_80[:], onesd[:, 0:SIZE])
            ones1_128 = cpool.tile([1, 128], F32R, tag="ones1_128")
            nc.sync.dma_start(ones1_128[:], onesd[:, 0:128])
            sel = [cpool.tile([TT, SIZE], BF16, tag=f"sel{p}", name=f"sel{p}")
                   for p in (0, 1)]
            nc.sync.dma_start(sel[0][:], seld[0])
            nc.sync.dma_start(sel[1][:], seld[1])
            bsb = [cpool.tile([1, MM], F32R, tag=f"b{s}", name=f"b{s}")
                   for s in (0, 1)]
            nc.sync.dma_start(bsb[0][:], b0r[:])
            nc.sync.dma_start(bsb[1][:], b1r[:])
            bosb = cpool.tile([1, OUT], F32R, tag="bo")
            nc.sync.dma_start(bosb[:], boutr[:])

            # ---------------- stage 1: h = x @ W.T + b, as hT chunks ------
            h_tiles = [[None] * CHUNKS, [None] * CHUNKS]
            with (
                tc.tile_pool(name="xpool", bufs=1) as xpool,
                tc.tile_pool(name="wpool", bufs=3) as wpool,
                tc.tile_pool(name="ps1", bufs=1, space="PSUM") as ps1,
            ):
                for side in (0, 1):
                    xsb = xpool.tile([128, KD * BC], F32R, tag="x")
                    for k in range(KD):
                        nc.sync.dma_start(xsb[:, ts(k, BC)],
                                          xdr[side][ts(k, 128), :])
                    for cg in range(4):  # chunk groups of 5
                        pss = [ps1.tile([SIZE, BC], F32, tag=f"s1_{g}", name=f"s1_{g}")
                               for g in range(5)]
                        for k in range(KD):
                            wk = wpool.tile([128, 5 * SIZE], F32R, tag="wk")
                            nc.sync.dma_start(
                                wk[:], wdr[side][ts(k, 128), ts(cg, 5 * SIZE)])
                            for g in range(5):
                                nc.tensor.matmul(
                                    pss[g][:], wk[:, ts(g, SIZE)],
                                    xsb[:, ts(k, BC)],
                                    start=(k == 0), stop=False)
                        for g in range(5):
                            c = cg * 5 + g
                            nc.tensor.matmul(
                                pss[g][:], bsb[side][0:1, ts(c, SIZE)],
                                ones512[:], start=False, stop=True)
                            ht = hpool.tile([SIZE + 1, BC], F32R,
                                            tag=f"h{side}_{c}")
                            nc.any.tensor_copy(ht[0:SIZE, :], pss[g][:])
                            nc.sync.dma_start(ht[SIZE:SIZE + 1, :], onesd[:])
                            h_tiles[side][c] = ht

            # ---------------- stage 2: chunk linears + bilinear merge -----
            zn_tiles = [None] * CHUNKS
            with (
                tc.tile_pool(name="mwpool", bufs=2) as mwpool,
                tc.tile_pool(name="wbpool", bufs=2) as wbpool,
                tc.tile_pool(name="spool", bufs=2) as spool,
                tc.tile_pool(name="rowpool", bufs=2) as rowpool,
                tc.tile_pool(name="ypsum", bufs=2, space="PSUM") as ypsum,
                tc.tile_pool(name="zpsum", bufs=2, space="PSUM") as zpsum,
                tc.tile_pool(name="npsum", bufs=1, space="PSUM") as npsum,
                tc.tile_pool(name="rbpsum", bufs=1, space="PSUM") as rbpsum,
            ):
                for c in range(CHUNKS):
                    mwsb = []
                    for side in (0, 1):
                        m = mwpool.tile([SIZE + 1, TDIM], F32R, tag=f"mw{side}")
                        nc.sync.dma_start(m[:], mwdr[side][c])
                        mwsb.append(m)
                    z_ps = zpsum.tile([SIZE, BC], F32, tag="z")
                    for j in range(NTT):
                        yb = []
                        for side in (0, 1):
                            y = ypsum.tile([TT, BC], F32, tag=f"y{side}")
                            nc.tensor.matmul(
                                y[:], mwsb[side][:, ts(j, TT)],
                                h_tiles[side][c][:],
                                start=True, stop=True)
                            b = wbpool.tile([TT, BC], BF16, tag=f"y{side}b")
                            nc.any.tensor_copy(b[:], y[:])
                            yb.append(b)
                        wb = wbpool.tile([TT, BC], BF16, tag="wb")
                        nc.vector.tensor_mul(wb[:], yb[0][:], yb[1][:])
                        nc.tensor.matmul(z_ps[:], sel[j % 2][:], wb[:],
                                         start=(j == 0), stop=(j == NTT - 1))
                    # signed sqrt + chunk L2 norm
                    a = spool.tile([SIZE, BC], F32R, tag="a")
                    nc.scalar.activation(a[:], z_ps[:], AF.Abs)
                    sg = spool.tile([SIZE, BC], F32, tag="sg")
                    nc.scalar.activation(sg[:], z_ps[:], AF.Sign)
                    sq = spool.tile([SIZE, BC], F32, tag="sq")
                    nc.scalar.activation(sq[:], a[:], AF.Sqrt)
                    zs = spool.tile([SIZE, BC], F32, tag="zs")
                    nc.vector.tensor_mul(zs[:], sg[:], sq[:])
                    n2 = npsum.tile([1, BC], F32, tag="n2")
                    nc.tensor.matmul(n2[:], ones80[:], a[:],
                                     start=True, stop=True)
                    sn = rowpool.tile([1, BC], F32, tag="sn")
                    nc.scalar.activation(sn[:], n2[:], AF.Sqrt)
                    nc.vector.tensor_scalar_max(sn[:], sn[:], EPS)
                    rn = rowpool.tile([1, BC], F32, tag="rn")
                    nc.vector.reciprocal_approx_fast(rn[:], sn[:])
                    rnr = rowpool.tile([1, BC], F32R, tag="rnr")
                    nc.vector.tensor_copy(rnr[:], rn[:])
                    rnb = rbpsum.tile([SIZE, BC], F32, tag="rnb")
                    nc.tensor.matmul(rnb[:], ones1_80[:], rnr[:],
                                     start=True, stop=True)
                    zn = hpool.tile([SIZE, BC], F32R, tag=f"h0_{c}")
                    nc.vector.tensor_mul(zn[:], zs[:], rnb[:])
                    zn_tiles[c] = zn

            # ---------------- stage 3: out = zn @ Wout.T + bout ----------
            with (
                tc.tile_pool(name="wopool", bufs=1) as wopool,
                tc.tile_pool(name="opool", bufs=2) as opool,
                tc.tile_pool(name="ps3", bufs=2, space="PSUM") as ps3,
            ):
                for nt in range(NNT):
                    wo = []
                    for c in range(CHUNKS):
                        w = wopool.tile([SIZE, NO], F32R, tag=f"wo_{c}")
                        nc.sync.dma_start(
                            w[:], wot[ts(c, SIZE), ts(nt, NO)])
                        wo.append(w)
                    for bt in range(NBT):
                        ops = ps3.tile([128, NO], F32, tag="o")
                        for c in range(CHUNKS):
                            nc.tensor.matmul(
                                ops[:], zn_tiles[c][:, ts(bt, 128)],
                                wo[c][:], start=(c == 0), stop=False)
                        nc.tensor.matmul(
                            ops[:], ones1_128[:],
                            bosb[0:1, ts(nt, NO)], start=False, stop=True)
                        osb = opool.tile([128, NO], F32, tag="ob")
                        nc.any.tensor_copy(osb[:], ops[:])
                        nc.sync.dma_start(outd[ts(bt, 128), ts(nt, NO)],
                                          osb[:])

    nc.compile()
    return nc


def _get_nc():
    global _NC
    if _NC is None:
        _NC = _build_nc()
    return _NC


def _prep_inputs(x0, x1, W0, b0, W1, b1, mW0, mb0, mW1, mb1, Wout, bout):
    import ml_dtypes
    f = np.float32
    shared = {
        "w0t": np.ascontiguousarray(W0.T, dtype=f),
        "w1t": np.ascontiguousarray(W1.T, dtype=f),
        "mw0p": np.ascontiguousarray(
            np.concatenate([mW0.transpose(0, 2, 1), mb0[:, None, :]], axis=1),
            dtype=f),
        "mw1p": np.ascontiguousarray(
            np.concatenate([mW1.transpose(0, 2, 1), mb1[:, None, :]], axis=1),
            dtype=f),
        "wot": np.ascontiguousarray(Wout.T, dtype=f),
        "b0r": np.ascontiguousarray(b0.reshape(1, MM), dtype=f),
        "b1r": np.ascontiguousarray(b1.reshape(1, MM), dtype=f),
        "boutr": np.ascontiguousarray(bout.reshape(1, OUT), dtype=f),
    }
    k = np.arange(TT)
    s = np.arange(SIZE)
    selm = np.stack([
        ((40 * p + k[:, None]) % SIZE == s[None, :]) for p in (0, 1)
    ]).astype(ml_dtypes.bfloat16)
    shared["seld"] = selm
    shared["onesd"] = np.ones((1, BC), dtype=f)
    shared["onescol"] = np.ones((128, 1), dtype=f)
    x0t = np.ascontiguousarray(x0.T, dtype=f)
    x1t = np.ascontiguousarray(x1.T, dtype=f)
    in_maps = []
    for c in range(NCORES):
        m = dict(shared)
        m["x0t"] = np.ascontiguousarray(x0t[:, c * BC:(c + 1) * BC])
        m["x1t"] = np.ascontiguousarray(x1t[:, c * BC:(c + 1) * BC])
        in_maps.append(m)
    return in_maps


_RUNNER = None


def _get_runner():
    """Build the sharded PJRT executable once and reuse it across calls."""
    global _RUNNER
    if _RUNNER is not None:
        return _RUNNER
    import jax
    from jax.sharding import Mesh, NamedSharding, PartitionSpec
    from jax.experimental.shard_map import shard_map
    from concourse.bass2jax import (
        _bass_exec_p, install_neuronx_cc_hook, partition_id_tensor)

    nc = _get_nc()
    install_neuronx_cc_hook()

    in_names, out_names, out_avals, zero_outs = [], [], [], []
    pname = nc.partition_id_tensor.name if nc.partition_id_tensor else None
    for alloc in nc.m.functions[0].allocations:
        if not isinstance(alloc, mybir.MemoryLocationSet):
            continue
        name = alloc.memorylocations[0].name
        if alloc.kind == "ExternalInput":
            if name != pname:
                in_names.append(name)
        elif alloc.kind == "ExternalOutput":
            shape = tuple(alloc.tensor_shape)
            dtype = mybir.dt.np(alloc.dtype)
            out_names.append(name)
            out_avals.append(jax.core.ShapedArray(shape, dtype))
            zero_outs.append(np.zeros(shape, dtype))
    n_params = len(in_names)
    all_names = in_names + out_names
    if pname is not None:
        all_names.append(pname)

    def _body(*args):
        operands = list(args)
        if pname is not None:
            operands.append(partition_id_tensor())
        return tuple(_bass_exec_p.bind(
            *operands,
            out_avals=tuple(out_avals),
            in_names=tuple(all_names),
            out_names=tuple(out_names),
            lowering_input_output_aliases=(),
            sim_require_finite=True,
            sim_require_nnan=True,
            nc=nc,
        ))

    devices = jax.devices()[:NCORES]
    mesh = Mesh(np.asarray(devices), ("core",))
    nin = n_params + len(out_names)
    fn = jax.jit(
        shard_map(_body, mesh=mesh,
                  in_specs=(PartitionSpec("core"),) * nin,
                  out_specs=(PartitionSpec("core"),) * len(out_names),
                  check_rep=False),
        keep_unused=True)
    sharding = NamedSharding(mesh, PartitionSpec("core"))
    zeros_dev = [jax.device_put(
        np.concatenate([z] * NCORES, axis=0), sharding) for z in zero_outs]
    _RUNNER = (fn, in_names, out_names, zeros_dev, sharding)
    return _RUNNER


def _put_inputs(in_maps):
    import jax
    fn, in_names, out_names, zeros_dev, sharding = _get_runner()
    return [jax.device_put(
        np.concatenate([in_maps[c][n] for c in range(NCORES)], axis=0),
        sharding) for n in in_names]


def _run(dev_in):
    import jax
    fn, in_names, out_names, zeros_dev, sharding = _get_runner()
    outs = fn(*dev_in, *zeros_dev)
    jax.block_until_ready(outs)
    return outs


def kernel(**inputs):
    inputs = {k: np.asarray(v, dtype=np.float32) for k, v in inputs.items()}
    in_maps = _prep_inputs(**inputs)
    dev_in = _put_inputs(in_maps)
    outs = _run(dev_in)
    full = np.asarray(outs[0])          # [NCORES*BC, OUT] concat over cores
    return np.ascontiguousarray(full.reshape(B, OUT))
